# revision 16
# baseline (speedup 1.0000x reference)
"""AttentionReadout kernel for 8 Trainium2 NeuronCores.

Math (reference): per-node k/v projections of x[N,512], per-head logits
s = (x@k_w.T + k_b) . q / sqrt(64), segment softmax over each graph's
nodes, weighted segment-sum of v, then o-projection + LayerNorm over
the [256, 512] graph outputs.

Key restructurings (all exact, up to fp reassociation):
  * Only k.q is needed, so the k-projection folds into W_s[512,8] =
    (k_w * q).heads.sum / 8 computed on host; s = x @ W_s.
  * Per-(graph,head) constants multiply both numerator and denominator
    of the softmax-average, so the segment-max subtraction and the
    k-bias term cancel -> w = exp(x @ W_s) directly (values are O(e^3),
    safe in fp32).
  * v-bias adds v_b * denom to the numerator -> attended = numer/denom
    + v_b; applied once in the tail.
  * The big reassociation: numer = segsum(w * (x @ v_w.T)) =
    (B.T @ x) @ v_w.T where B[n, (g,h)] = onehot[n,g] * w[n,h].
    Contracting nodes FIRST (256 output columns) costs half the PE work
    of projecting every node (512 columns), and the v-projection then
    runs once on the tiny [256, 512] aggregate in the tail.
  * B is built on-device from the segment ids (iota + is_equal) and the
    exp weights; the logits matmul runs transposed (s.T = W_s.T @ x.T)
    with a bf16 copy of x.T shipped from the host (softmax averaging
    washes out the bf16 logit rounding).

Sharding: batch is sorted, so core c owns graphs [32c, 32c+32) and their
contiguous node range, zero-padded to PAD_N.
"""

import numpy as np
from contextlib import ExitStack

import ml_dtypes
import concourse.bass as bass
import concourse.bacc as bacc
import concourse.tile as tile
from concourse import mybir
from concourse.bass_utils import run_bass_kernel_spmd
from concourse.masks import make_identity

N_CORES = 8
G = 256
G_LOC = G // N_CORES  # 32 graphs per core
H = 512
NH = 8
HD = 64
CHUNK = 512  # nodes per chunk
PAD_N = 13312  # 26 chunks; actual max per-core nodes is 12653 for this problem size
NCHUNK = PAD_N // CHUNK
KC = H // 128  # 4 contraction sub-chunks
JC = CHUNK // 128  # 4 node sub-chunks per chunk
GH = G_LOC * NH  # 256 (graph, head) columns
NHP = 16  # logits padded to 16 rows for the DMA-transpose xbar (src rows % 16)
LN_EPS = 1e-5

F32 = mybir.dt.float32
F32R = mybir.dt.float32r
BF16 = mybir.dt.bfloat16
FP16 = mybir.dt.float16


def _bcast_rows(ap_1d, parts):
    """[D] dram AP -> [parts, D] partition-broadcast AP (stride-0 partitions)."""
    return bass.AP(tensor=ap_1d.tensor, offset=ap_1d.offset, ap=[[0, parts]] + list(ap_1d.ap))


def _dup_inner(ap, n):
    """Append a 0-stride length-n innermost dim (free-dim broadcast)."""
    return bass.AP(tensor=ap.tensor, offset=ap.offset, ap=list(ap.ap) + [[0, n]])


def _dup_mid(ap, n):
    """Insert a 0-stride length-n dim before the innermost free dim."""
    aps = list(ap.ap)
    return bass.AP(tensor=ap.tensor, offset=ap.offset, ap=aps[:-1] + [[0, n]] + aps[-1:])


def build_bass():
    nc = bacc.Bacc(None)

    xn = nc.declare_dram_parameter("xn", [NCHUNK, CHUNK, H], F32R, isOutput=False)
    xtb = nc.declare_dram_parameter("xtb", [NCHUNK, H, CHUNK], FP16, isOutput=False)
    bt = nc.declare_dram_parameter("bt", [128, NCHUNK, JC], F32, isOutput=False)
    vwt = nc.declare_dram_parameter("vwt", [H, H], F32R, isOutput=False)
    wst = nc.declare_dram_parameter("wst", [H, NHP], FP16, isOutput=False)
    owt = nc.declare_dram_parameter("owt", [H, H], F32R, isOutput=False)
    vb = nc.declare_dram_parameter("vb", [H], F32, isOutput=False)
    ob = nc.declare_dram_parameter("ob", [H], F32, isOutput=False)
    lng = nc.declare_dram_parameter("lng", [H], F32, isOutput=False)
    lnb = nc.declare_dram_parameter("lnb", [H], F32, isOutput=False)
    out = nc.declare_dram_parameter("out", [G_LOC, H], F32, isOutput=True)

    xn_r = xn.rearrange("c (j p) f -> c p j f", p=128)
    xtb_r = xtb.rearrange("c (k p) n -> c p k n", p=128)
    vwt_r = vwt.rearrange("(k p) o -> p k o", p=128)
    wst_r = wst.rearrange("(k p) h -> p k h", p=128)
    owt_r = owt.rearrange("(k p) o -> p k o", p=128)

    with tile.TileContext(nc) as tc, ExitStack() as ctx:
        consts = ctx.enter_context(tc.tile_pool(name="consts", bufs=1))
        acc_psum = ctx.enter_context(tc.tile_pool(name="acc_psum", bufs=1, space="PSUM"))

        # --- constants (loaded once) ---
        vwt_sb = consts.tile([128, KC, H], F32R)
        nc.sync.dma_start(out=vwt_sb, in_=vwt_r)
        wst_sb = consts.tile([128, KC, NHP], FP16)
        nc.sync.dma_start(out=wst_sb, in_=wst_r)
        owt_sb = consts.tile([128, KC, H], F32R)
        nc.sync.dma_start(out=owt_sb, in_=owt_r)
        bt_sb = consts.tile([128, NCHUNK, JC], F32)
        nc.sync.dma_start(out=bt_sb, in_=bt[:, :, :])
        vb_sb = consts.tile([G_LOC, H], F32)
        nc.gpsimd.dma_start(out=vb_sb, in_=_bcast_rows(vb[:], G_LOC))
        ob_sb = consts.tile([G_LOC, H], F32)
        nc.gpsimd.dma_start(out=ob_sb, in_=_bcast_rows(ob[:], G_LOC))
        lng_sb = consts.tile([G_LOC, H], F32)
        nc.gpsimd.dma_start(out=lng_sb, in_=_bcast_rows(lng[:], G_LOC))
        lnb_sb = consts.tile([G_LOC, H], F32)
        nc.gpsimd.dma_start(out=lnb_sb, in_=_bcast_rows(lnb[:], G_LOC))

        iota_g = consts.tile([128, G_LOC], F32)
        nc.gpsimd.iota(iota_g, pattern=[[1, G_LOC]], base=0, channel_multiplier=0,
                       allow_small_or_imprecise_dtypes=True)
        identity = consts.tile([128, 128], F32)
        make_identity(nc, identity)
        eps_t = consts.tile([G_LOC, 1], F32)
        nc.vector.memset(eps_t, LN_EPS)

        # cross-chunk accumulators: C = B.T @ x (two 128-col halves), denom
        c0_ps = acc_psum.tile([128, H], F32)
        c1_ps = acc_psum.tile([128, H], F32)
        denom_ps = acc_psum.tile([G_LOC, NH], F32)
        c_halves = [c0_ps, c1_ps]

        # --- main loop over node chunks ---
        with ExitStack() as loop_ctx:
            xn_pool = loop_ctx.enter_context(tc.tile_pool(name="xn", bufs=3))
            xtb_pool = loop_ctx.enter_context(tc.tile_pool(name="xtb", bufs=3))
            work = loop_ctx.enter_context(tc.tile_pool(name="work", bufs=2))
            sT_pool = loop_ctx.enter_context(tc.tile_pool(name="sT_ps", bufs=2, space="PSUM"))

            def emit_s_matmuls(i):
                # s.T = W_s.T @ x.T : one [8, 512] fp16 matmul chain per chunk,
                # emitted one chunk ahead; the PSUM->SBUF fp16 copy and the
                # [8,128]->[128,8] DMA transposes (scalar-engine HWDGE, so the
                # xbar stays in transpose mode on those queues) overlap C-work
                xtb_sb = xtb_pool.tile([128, KC, CHUNK], FP16, tag="xtb")
                nc.sync.dma_start(out=xtb_sb, in_=xtb_r[i])
                sT_ps = sT_pool.tile([NHP, CHUNK], F32, tag="sT")
                sT_f16 = work.tile([NHP, CHUNK], FP16, tag="sT16")
                sn_sb = work.tile([128, JC, NHP], FP16, tag="snsb")
                for k in range(KC):
                    nc.tensor.matmul(sT_ps, lhsT=wst_sb[:, k, :], rhs=xtb_sb[:, k, :],
                                     start=(k == 0), stop=(k == KC - 1))
                nc.scalar.copy(sT_f16, sT_ps)
                for j in range(JC):
                    nc.scalar.dma_start(out=sn_sb[:, j, :], in_=sT_f16[:, bass.ts(j, 128)],
                                        transpose=True)
                return sn_sb

            sT_next = emit_s_matmuls(0)
            for i in range(NCHUNK):
                xn_sb = xn_pool.tile([128, JC, H], F32R)
                nc.sync.dma_start(out=xn_sb, in_=xn_r[i])

                at_sb = work.tile([128, JC, G_LOC], F32R)
                for j in range(JC):
                    b1 = bt_sb[:, i, j:j + 1]
                    bt_b = bass.AP(tensor=b1.tensor, offset=b1.offset,
                                   ap=[list(b1.ap[0])] + [[0, G_LOC]])
                    nc.vector.tensor_tensor(out=at_sb[:, j, :], in0=iota_g, in1=bt_b,
                                            op=mybir.AluOpType.is_equal)

                sn_sb = sT_next
                w_n = work.tile([128, JC, NH], F32R)
                bmat = work.tile([128, JC, G_LOC, NH], F32R)

                for j in range(JC):
                    nc.scalar.activation(w_n[:, j, :], sn_sb[:, j, :NH],
                                         mybir.ActivationFunctionType.Exp)
                    # B[n, (g,h)] = at[n,g] * w[n,h]
                    nc.vector.tensor_tensor(
                        out=bmat[:, j, :, :],
                        in0=_dup_inner(at_sb[:, j, :], NH),
                        in1=_dup_mid(w_n[:, j, :], G_LOC),
                        op=mybir.AluOpType.mult)

                if i + 1 < NCHUNK:
                    sT_next = emit_s_matmuls(i + 1)

                for j in range(JC):
                    bflat = bmat[:, j, :, :].rearrange("p g h -> p (g h)")
                    for half in range(2):
                        nc.tensor.matmul(
                            c_halves[half], lhsT=bflat[:, bass.ts(half, 128)],
                            rhs=xn_sb[:, j, :],
                            start=(i == 0 and j == 0), stop=(i == NCHUNK - 1 and j == JC - 1),
                            skip_group_check=True)
                    nc.tensor.matmul(
                        denom_ps, lhsT=at_sb[:, j, :], rhs=w_n[:, j, :],
                        start=(i == 0 and j == 0), stop=(i == NCHUNK - 1 and j == JC - 1),
                        skip_group_check=True)

        # --- tail: C -> C.T -> numer -> attended -> o-projection -> layernorm ---
        with ExitStack() as tail_ctx:
            tail = tail_ctx.enter_context(tc.tile_pool(name="tail", bufs=1))
            tail_ps = tail_ctx.enter_context(tc.tile_pool(name="tail_ps", bufs=2, space="PSUM"))
            acc2_ps = tail_ctx.enter_context(tc.tile_pool(name="acc2_ps", bufs=1, space="PSUM"))

            c_sb = tail.tile([128, 2, H], F32)
            nc.vector.tensor_copy(c_sb[:, 0, :], c0_ps)
            nc.scalar.copy(c_sb[:, 1, :], c1_ps)

            # transpose C [256(gh), 512(i)] -> CT [512(i), 256(gh)]
            ct_sb = tail.tile([128, KC, GH], F32R)
            for k in range(KC):
                for half in range(2):
                    ctp = tail_ps.tile([128, 128], F32, tag="ctp")
                    nc.tensor.transpose(ctp, c_sb[:, half, bass.ts(k, 128)],
                                        identity[:, :])
                    eng = nc.vector.tensor_copy if half == 0 else nc.scalar.copy
                    eng(ct_sb[:, k, bass.ts(half, 128)], ctp)

            # numer[(g,h), :] = CT_h.T @ vwt[:, h*64:(h+1)*64]
            numer_ps = acc2_ps.tile([G_LOC, NH, HD], F32)
            ct_ghview = ct_sb.rearrange("p k (g h) -> p k g h", h=NH)
            for h in range(NH):
                for k in range(KC):
                    nc.tensor.matmul(
                        numer_ps[:, h, :], lhsT=ct_ghview[:, k, :, h],
                        rhs=vwt_sb[:, k, bass.ts(h, HD)],
                        start=(k == 0), stop=(k == KC - 1))

            recip = tail.tile([G_LOC, NH], F32)
            nc.vector.reciprocal(recip, denom_ps)

            att = tail.tile([G_LOC, NH, HD], F32)
            nc.vector.tensor_tensor(out=att[:, :, :],
                                    in0=numer_ps[:, :, :],
                                    in1=_dup_inner(recip[:, :], HD),
                                    op=mybir.AluOpType.mult)
            att_f = att.rearrange("g h d -> g (h d)")
            nc.vector.tensor_add(att_f, att_f, vb_sb)

            attT_ps = acc2_ps.tile([128, KC, G_LOC], F32)
            attT_sb = tail.tile([128, KC, G_LOC], F32R)
            for k in range(KC):
                nc.tensor.transpose(attT_ps[:, k, :], att_f[:, bass.ts(k, 128)],
                                    identity[:G_LOC, :G_LOC])
            nc.scalar.copy(attT_sb.rearrange("p k g -> p (k g)"),
                           attT_ps.rearrange("p k g -> p (k g)"))

            oo_ps = acc2_ps.tile([G_LOC, H], F32)
            for k in range(KC):
                nc.tensor.matmul(oo_ps, lhsT=attT_sb[:, k, :], rhs=owt_sb[:, k, :],
                                 start=(k == 0), stop=(k == KC - 1))

            t_sb = tail.tile([G_LOC, H], F32)
            nc.vector.tensor_add(t_sb, oo_ps, ob_sb)
            stats = tail.tile([G_LOC, 6], F32)
            nc.vector.bn_stats(out=stats, in_=t_sb)
            mv = tail.tile([G_LOC, 2], F32)
            nc.vector.bn_aggr(out=mv, in_=stats)
            std = tail.tile([G_LOC, 1], F32)
            nc.scalar.activation(std, mv[:, 1:2], mybir.ActivationFunctionType.Sqrt,
                                 bias=eps_t[:, 0:1])
            rstd = tail.tile([G_LOC, 1], F32)
            nc.vector.reciprocal(rstd, std)

            m1 = mv[:, 0:1]
            mean_b = bass.AP(tensor=m1.tensor, offset=m1.offset,
                             ap=[list(m1.ap[0])] + [[0, H]])
            r1 = rstd[:, 0:1]
            rstd_b = bass.AP(tensor=r1.tensor, offset=r1.offset,
                             ap=[list(r1.ap[0])] + [[0, H]])
            t2 = tail.tile([G_LOC, H], F32)
            nc.vector.tensor_sub(t2, t_sb, mean_b)
            nc.vector.tensor_mul(t2, t2, rstd_b)
            nc.vector.tensor_mul(t2, t2, lng_sb)
            out_sb = tail.tile([G_LOC, H], F32)
            nc.vector.tensor_add(out_sb, t2, lnb_sb)
            nc.sync.dma_start(out=out[:, :], in_=out_sb)

    nc.finalize()
    return nc


_NC_CACHE = {}


def _get_nc():
    if "nc" not in _NC_CACHE:
        _NC_CACHE["nc"] = build_bass()
    return _NC_CACHE["nc"]


def kernel(x, batch, query, k_w, k_b, v_w, v_b, o_w, o_b, ln_g, ln_b, num_graphs):
    x = np.ascontiguousarray(np.asarray(x, dtype=np.float32))
    batch = np.asarray(batch).astype(np.int64)
    query = np.asarray(query, dtype=np.float32)
    k_w = np.asarray(k_w, dtype=np.float32)
    v_w = np.asarray(v_w, dtype=np.float32)
    o_w = np.asarray(o_w, dtype=np.float32)
    v_b = np.asarray(v_b, dtype=np.float32)
    o_b = np.asarray(o_b, dtype=np.float32)
    ln_g = np.asarray(ln_g, dtype=np.float32)
    ln_b = np.asarray(ln_b, dtype=np.float32)

    # fold k-projection + query dot + 1/sqrt(hd) into one [H, NH] matrix
    w_s = np.zeros((H, NHP), np.float16)
    w_s[:, :NH] = ((k_w * query[:, None]).reshape(NH, HD, H).sum(1).T
                   / np.float32(np.sqrt(HD))).astype(np.float16)
    vwt_h = np.ascontiguousarray(v_w.T)
    owt_h = np.ascontiguousarray(o_w.T)

    bounds = np.searchsorted(batch, np.arange(0, G + 1, G_LOC))
    in_maps = []
    for c in range(N_CORES):
        s, e = int(bounds[c]), int(bounds[c + 1])
        n_loc = e - s
        assert n_loc <= PAD_N, f"shard {c} has {n_loc} nodes > PAD_N={PAD_N}"
        xs = np.zeros((PAD_N, H), np.float32)
        xs[:n_loc] = x[s:e]
        xn_h = np.ascontiguousarray(xs.reshape(NCHUNK, CHUNK, H))
        xtb_h = np.ascontiguousarray(
            xs.reshape(NCHUNK, CHUNK, H).transpose(0, 2, 1)).astype(np.float16)
        bl = np.full(PAD_N, -1.0, np.float32)
        bl[:n_loc] = (batch[s:e] - G_LOC * c).astype(np.float32)
        bt_h = np.ascontiguousarray(bl.reshape(NCHUNK, JC, 128).transpose(2, 0, 1))
        in_maps.append(dict(xn=xn_h, xtb=xtb_h, bt=bt_h, vwt=vwt_h, wst=w_s, owt=owt_h,
                            vb=v_b, ob=o_b, lng=ln_g, lnb=ln_b))

    nc = _get_nc()
    res = run_bass_kernel_spmd(nc, in_maps, core_ids=list(range(N_CORES)))
    _NC_CACHE["last_results"] = res
    return np.concatenate([res.results[c]["out"] for c in range(N_CORES)], axis=0)


# revision 17
# speedup vs baseline: 1.8237x; 1.8237x over previous
"""AttentionReadout kernel for 8 Trainium2 NeuronCores.

Math (reference): per-node k/v projections of x[N,512], per-head logits
s = (x@k_w.T + k_b) . q / sqrt(64), segment softmax over each graph's
nodes, weighted segment-sum of v, then o-projection + LayerNorm over
the [256, 512] graph outputs.

Key restructurings (all exact, up to fp reassociation):
  * Only k.q is needed, so the k-projection folds into W_s[512,8] =
    (k_w * q).heads.sum / 8 computed on host; s = x @ W_s.
  * Per-(graph,head) constants multiply both numerator and denominator
    of the softmax-average, so the segment-max subtraction and the
    k-bias term cancel -> w = exp(x @ W_s) directly (values are O(e^3),
    safe in fp32).
  * v-bias adds v_b * denom to the numerator -> attended = numer/denom
    + v_b; applied once in the tail.
  * The big reassociation: numer = segsum(w * (x @ v_w.T)) =
    (B.T @ x) @ v_w.T where B[n, (g,h)] = onehot[n,g] * w[n,h].
    Contracting nodes FIRST (256 output columns) costs half the PE work
    of projecting every node (512 columns), and the v-projection then
    runs once on the tiny [256, 512] aggregate in the tail.
  * B is built on-device from the segment ids (iota + is_equal) and the
    exp weights; the logits matmul runs transposed (s.T = W_s.T @ x.T)
    with a bf16 copy of x.T shipped from the host (softmax averaging
    washes out the bf16 logit rounding).

Sharding: batch is sorted, so core c owns graphs [32c, 32c+32) and their
contiguous node range, zero-padded to PAD_N.
"""

import numpy as np
from contextlib import ExitStack

import ml_dtypes
import concourse.bass as bass
import concourse.bacc as bacc
import concourse.tile as tile
from concourse import mybir
from concourse.bass_utils import run_bass_kernel_spmd
from concourse.masks import make_identity

N_CORES = 8
G = 256
G_LOC = G // N_CORES  # 32 graphs per core
H = 512
NH = 8
HD = 64
CHUNK = 512  # nodes per chunk
PAD_N = 13312  # 26 chunks; actual max per-core nodes is 12653 for this problem size
NCHUNK = PAD_N // CHUNK
KC = H // 128  # 4 contraction sub-chunks
JC = CHUNK // 128  # 4 node sub-chunks per chunk
GH = G_LOC * NH  # 256 (graph, head) columns
NHP = 16  # logits padded to 16 rows for the DMA-transpose xbar (src rows % 16)
LN_EPS = 1e-5

F32 = mybir.dt.float32
F32R = mybir.dt.float32r
BF16 = mybir.dt.bfloat16
FP16 = mybir.dt.float16


def _bcast_rows(ap_1d, parts):
    """[D] dram AP -> [parts, D] partition-broadcast AP (stride-0 partitions)."""
    return bass.AP(tensor=ap_1d.tensor, offset=ap_1d.offset, ap=[[0, parts]] + list(ap_1d.ap))


def _dup_inner(ap, n):
    """Append a 0-stride length-n innermost dim (free-dim broadcast)."""
    return bass.AP(tensor=ap.tensor, offset=ap.offset, ap=list(ap.ap) + [[0, n]])


def _dup_mid(ap, n):
    """Insert a 0-stride length-n dim before the innermost free dim."""
    aps = list(ap.ap)
    return bass.AP(tensor=ap.tensor, offset=ap.offset, ap=aps[:-1] + [[0, n]] + aps[-1:])


def build_bass():
    nc = bacc.Bacc(None)

    xn = nc.declare_dram_parameter("xn", [NCHUNK, CHUNK, H], F32R, isOutput=False)
    xtb = nc.declare_dram_parameter("xtb", [NCHUNK, H, CHUNK], FP16, isOutput=False)
    bt = nc.declare_dram_parameter("bt", [128, NCHUNK, JC], F32, isOutput=False)
    vwt = nc.declare_dram_parameter("vwt", [H, H], F32R, isOutput=False)
    wst = nc.declare_dram_parameter("wst", [H, NHP], FP16, isOutput=False)
    owt = nc.declare_dram_parameter("owt", [H, H], F32R, isOutput=False)
    vb = nc.declare_dram_parameter("vb", [H], F32, isOutput=False)
    ob = nc.declare_dram_parameter("ob", [H], F32, isOutput=False)
    lng = nc.declare_dram_parameter("lng", [H], F32, isOutput=False)
    lnb = nc.declare_dram_parameter("lnb", [H], F32, isOutput=False)
    out = nc.declare_dram_parameter("out", [G_LOC, H], F32, isOutput=True)

    xn_r = xn.rearrange("c (j p) f -> c p j f", p=128)
    xtb_r = xtb.rearrange("c (k p) n -> c p k n", p=128)
    vwt_r = vwt.rearrange("(k p) o -> p k o", p=128)
    wst_r = wst.rearrange("(k p) h -> p k h", p=128)
    owt_r = owt.rearrange("(k p) o -> p k o", p=128)

    with tile.TileContext(nc) as tc, ExitStack() as ctx:
        consts = ctx.enter_context(tc.tile_pool(name="consts", bufs=1))
        acc_psum = ctx.enter_context(tc.tile_pool(name="acc_psum", bufs=1, space="PSUM"))

        # --- constants (loaded once) ---
        vwt_sb = consts.tile([128, KC, H], F32R)
        nc.sync.dma_start(out=vwt_sb, in_=vwt_r)
        wst_sb = consts.tile([128, KC, NHP], FP16)
        nc.sync.dma_start(out=wst_sb, in_=wst_r)
        owt_sb = consts.tile([128, KC, H], F32R)
        nc.sync.dma_start(out=owt_sb, in_=owt_r)
        bt_sb = consts.tile([128, NCHUNK, JC], F32)
        nc.sync.dma_start(out=bt_sb, in_=bt[:, :, :])
        vb_sb = consts.tile([G_LOC, H], F32)
        nc.gpsimd.dma_start(out=vb_sb, in_=_bcast_rows(vb[:], G_LOC))
        ob_sb = consts.tile([G_LOC, H], F32)
        nc.gpsimd.dma_start(out=ob_sb, in_=_bcast_rows(ob[:], G_LOC))
        lng_sb = consts.tile([G_LOC, H], F32)
        nc.gpsimd.dma_start(out=lng_sb, in_=_bcast_rows(lng[:], G_LOC))
        lnb_sb = consts.tile([G_LOC, H], F32)
        nc.gpsimd.dma_start(out=lnb_sb, in_=_bcast_rows(lnb[:], G_LOC))

        iota_g = consts.tile([128, G_LOC], F32)
        nc.gpsimd.iota(iota_g, pattern=[[1, G_LOC]], base=0, channel_multiplier=0,
                       allow_small_or_imprecise_dtypes=True)
        identity = consts.tile([128, 128], F32)
        make_identity(nc, identity)
        eps_t = consts.tile([G_LOC, 1], F32)
        nc.vector.memset(eps_t, LN_EPS)

        # cross-chunk accumulators: C = B.T @ x (two 128-col halves), denom
        c0_ps = acc_psum.tile([128, H], F32)
        c1_ps = acc_psum.tile([128, H], F32)
        denom_ps = acc_psum.tile([G_LOC, NH], F32)
        c_halves = [c0_ps, c1_ps]

        # --- main loop over node chunks ---
        with ExitStack() as loop_ctx:
            xn_pool = loop_ctx.enter_context(tc.tile_pool(name="xn", bufs=3))
            xtb_pool = loop_ctx.enter_context(tc.tile_pool(name="xtb", bufs=3))
            work = loop_ctx.enter_context(tc.tile_pool(name="work", bufs=2))
            sT_pool = loop_ctx.enter_context(tc.tile_pool(name="sT_ps", bufs=2, space="PSUM"))
            sn_pool = loop_ctx.enter_context(tc.tile_pool(name="sn_ps", bufs=2, space="PSUM"))

            def emit_s_matmuls(i):
                # s.T = W_s.T @ x.T : one [8, 512] fp16 matmul chain per chunk,
                # emitted one chunk ahead so the PSUM->SBUF copy overlaps C-work
                xtb_sb = xtb_pool.tile([128, KC, CHUNK], FP16, tag="xtb")
                nc.sync.dma_start(out=xtb_sb, in_=xtb_r[i])
                sT_ps = sT_pool.tile([NHP, CHUNK], F32, tag="sT")
                sT_sb = work.tile([NHP, CHUNK], F32, tag="sTsb")
                for k in range(KC):
                    nc.tensor.matmul(sT_ps, lhsT=wst_sb[:, k, :], rhs=xtb_sb[:, k, :],
                                     start=(k == 0), stop=(k == KC - 1))
                nc.scalar.copy(sT_sb, sT_ps)
                return sT_sb

            sT_next = emit_s_matmuls(0)
            for i in range(NCHUNK):
                xn_sb = xn_pool.tile([128, JC, H], F32R)
                nc.sync.dma_start(out=xn_sb, in_=xn_r[i])

                at_sb = work.tile([128, JC, G_LOC], F32R)
                for j in range(JC):
                    b1 = bt_sb[:, i, j:j + 1]
                    bt_b = bass.AP(tensor=b1.tensor, offset=b1.offset,
                                   ap=[list(b1.ap[0])] + [[0, G_LOC]])
                    nc.vector.tensor_tensor(out=at_sb[:, j, :], in0=iota_g, in1=bt_b,
                                            op=mybir.AluOpType.is_equal)

                sT_sb = sT_next
                sn_ps = sn_pool.tile([128, JC, NH], F32)
                w_n = work.tile([128, JC, NH], F32R)
                bmat = work.tile([128, JC, G_LOC, NH], F32R)

                # PE: transposes first (sT copy landed during last chunk's C),
                # then next chunk's s-matmuls, then this chunk's C matmuls.
                for j in range(JC):
                    nc.tensor.transpose(sn_ps[:, j, :], sT_sb[:NH, bass.ts(j, 128)],
                                        identity[:NH, :NH])
                for j in range(JC):
                    nc.scalar.activation(w_n[:, j, :], sn_ps[:, j, :],
                                         mybir.ActivationFunctionType.Exp)
                    # B[n, (g,h)] = at[n,g] * w[n,h]
                    nc.vector.tensor_tensor(
                        out=bmat[:, j, :, :],
                        in0=_dup_inner(at_sb[:, j, :], NH),
                        in1=_dup_mid(w_n[:, j, :], G_LOC),
                        op=mybir.AluOpType.mult)

                if i + 1 < NCHUNK:
                    sT_next = emit_s_matmuls(i + 1)

                for j in range(JC):
                    bflat = bmat[:, j, :, :].rearrange("p g h -> p (g h)")
                    for half in range(2):
                        nc.tensor.matmul(
                            c_halves[half], lhsT=bflat[:, bass.ts(half, 128)],
                            rhs=xn_sb[:, j, :],
                            start=(i == 0 and j == 0), stop=(i == NCHUNK - 1 and j == JC - 1),
                            skip_group_check=True)
                    nc.tensor.matmul(
                        denom_ps, lhsT=at_sb[:, j, :], rhs=w_n[:, j, :],
                        start=(i == 0 and j == 0), stop=(i == NCHUNK - 1 and j == JC - 1),
                        skip_group_check=True)

        # --- tail: C -> C.T -> numer -> attended -> o-projection -> layernorm ---
        with ExitStack() as tail_ctx:
            tail = tail_ctx.enter_context(tc.tile_pool(name="tail", bufs=1))
            tail_ps = tail_ctx.enter_context(tc.tile_pool(name="tail_ps", bufs=2, space="PSUM"))
            acc2_ps = tail_ctx.enter_context(tc.tile_pool(name="acc2_ps", bufs=1, space="PSUM"))

            c_sb = tail.tile([128, 2, H], F32)
            nc.vector.tensor_copy(c_sb[:, 0, :], c0_ps)
            nc.scalar.copy(c_sb[:, 1, :], c1_ps)

            # transpose C [256(gh), 512(i)] -> CT [512(i), 256(gh)]
            ct_sb = tail.tile([128, KC, GH], F32R)
            for k in range(KC):
                for half in range(2):
                    ctp = tail_ps.tile([128, 128], F32, tag="ctp")
                    nc.tensor.transpose(ctp, c_sb[:, half, bass.ts(k, 128)],
                                        identity[:, :])
                    eng = nc.vector.tensor_copy if half == 0 else nc.scalar.copy
                    eng(ct_sb[:, k, bass.ts(half, 128)], ctp)

            # numer[(g,h), :] = CT_h.T @ vwt[:, h*64:(h+1)*64]
            numer_ps = acc2_ps.tile([G_LOC, NH, HD], F32)
            ct_ghview = ct_sb.rearrange("p k (g h) -> p k g h", h=NH)
            for h in range(NH):
                for k in range(KC):
                    nc.tensor.matmul(
                        numer_ps[:, h, :], lhsT=ct_ghview[:, k, :, h],
                        rhs=vwt_sb[:, k, bass.ts(h, HD)],
                        start=(k == 0), stop=(k == KC - 1))

            recip = tail.tile([G_LOC, NH], F32)
            nc.vector.reciprocal(recip, denom_ps)

            att = tail.tile([G_LOC, NH, HD], F32)
            nc.vector.tensor_tensor(out=att[:, :, :],
                                    in0=numer_ps[:, :, :],
                                    in1=_dup_inner(recip[:, :], HD),
                                    op=mybir.AluOpType.mult)
            att_f = att.rearrange("g h d -> g (h d)")
            nc.vector.tensor_add(att_f, att_f, vb_sb)

            attT_ps = acc2_ps.tile([128, KC, G_LOC], F32)
            attT_sb = tail.tile([128, KC, G_LOC], F32R)
            for k in range(KC):
                nc.tensor.transpose(attT_ps[:, k, :], att_f[:, bass.ts(k, 128)],
                                    identity[:G_LOC, :G_LOC])
            nc.scalar.copy(attT_sb.rearrange("p k g -> p (k g)"),
                           attT_ps.rearrange("p k g -> p (k g)"))

            oo_ps = acc2_ps.tile([G_LOC, H], F32)
            for k in range(KC):
                nc.tensor.matmul(oo_ps, lhsT=attT_sb[:, k, :], rhs=owt_sb[:, k, :],
                                 start=(k == 0), stop=(k == KC - 1))

            t_sb = tail.tile([G_LOC, H], F32)
            nc.vector.tensor_add(t_sb, oo_ps, ob_sb)
            stats = tail.tile([G_LOC, 6], F32)
            nc.vector.bn_stats(out=stats, in_=t_sb)
            mv = tail.tile([G_LOC, 2], F32)
            nc.vector.bn_aggr(out=mv, in_=stats)
            std = tail.tile([G_LOC, 1], F32)
            nc.scalar.activation(std, mv[:, 1:2], mybir.ActivationFunctionType.Sqrt,
                                 bias=eps_t[:, 0:1])
            rstd = tail.tile([G_LOC, 1], F32)
            nc.vector.reciprocal(rstd, std)

            m1 = mv[:, 0:1]
            mean_b = bass.AP(tensor=m1.tensor, offset=m1.offset,
                             ap=[list(m1.ap[0])] + [[0, H]])
            r1 = rstd[:, 0:1]
            rstd_b = bass.AP(tensor=r1.tensor, offset=r1.offset,
                             ap=[list(r1.ap[0])] + [[0, H]])
            t2 = tail.tile([G_LOC, H], F32)
            nc.vector.tensor_sub(t2, t_sb, mean_b)
            nc.vector.tensor_mul(t2, t2, rstd_b)
            nc.vector.tensor_mul(t2, t2, lng_sb)
            out_sb = tail.tile([G_LOC, H], F32)
            nc.vector.tensor_add(out_sb, t2, lnb_sb)
            nc.sync.dma_start(out=out[:, :], in_=out_sb)

    nc.finalize()
    return nc


_NC_CACHE = {}


def _get_nc():
    if "nc" not in _NC_CACHE:
        _NC_CACHE["nc"] = build_bass()
    return _NC_CACHE["nc"]


def kernel(x, batch, query, k_w, k_b, v_w, v_b, o_w, o_b, ln_g, ln_b, num_graphs):
    x = np.ascontiguousarray(np.asarray(x, dtype=np.float32))
    batch = np.asarray(batch).astype(np.int64)
    query = np.asarray(query, dtype=np.float32)
    k_w = np.asarray(k_w, dtype=np.float32)
    v_w = np.asarray(v_w, dtype=np.float32)
    o_w = np.asarray(o_w, dtype=np.float32)
    v_b = np.asarray(v_b, dtype=np.float32)
    o_b = np.asarray(o_b, dtype=np.float32)
    ln_g = np.asarray(ln_g, dtype=np.float32)
    ln_b = np.asarray(ln_b, dtype=np.float32)

    # fold k-projection + query dot + 1/sqrt(hd) into one [H, NH] matrix
    w_s = np.zeros((H, NHP), np.float16)
    w_s[:, :NH] = ((k_w * query[:, None]).reshape(NH, HD, H).sum(1).T
                   / np.float32(np.sqrt(HD))).astype(np.float16)
    vwt_h = np.ascontiguousarray(v_w.T)
    owt_h = np.ascontiguousarray(o_w.T)

    bounds = np.searchsorted(batch, np.arange(0, G + 1, G_LOC))
    in_maps = []
    for c in range(N_CORES):
        s, e = int(bounds[c]), int(bounds[c + 1])
        n_loc = e - s
        assert n_loc <= PAD_N, f"shard {c} has {n_loc} nodes > PAD_N={PAD_N}"
        xs = np.zeros((PAD_N, H), np.float32)
        xs[:n_loc] = x[s:e]
        xn_h = np.ascontiguousarray(xs.reshape(NCHUNK, CHUNK, H))
        xtb_h = np.ascontiguousarray(
            xs.reshape(NCHUNK, CHUNK, H).transpose(0, 2, 1)).astype(np.float16)
        bl = np.full(PAD_N, -1.0, np.float32)
        bl[:n_loc] = (batch[s:e] - G_LOC * c).astype(np.float32)
        bt_h = np.ascontiguousarray(bl.reshape(NCHUNK, JC, 128).transpose(2, 0, 1))
        in_maps.append(dict(xn=xn_h, xtb=xtb_h, bt=bt_h, vwt=vwt_h, wst=w_s, owt=owt_h,
                            vb=v_b, ob=o_b, lng=ln_g, lnb=ln_b))

    nc = _get_nc()
    res = run_bass_kernel_spmd(nc, in_maps, core_ids=list(range(N_CORES)))
    _NC_CACHE["last_results"] = res
    return np.concatenate([res.results[c]["out"] for c in range(N_CORES)], axis=0)


# revision 18
# speedup vs baseline: 1.8246x; 1.0005x over previous
"""AttentionReadout kernel for 8 Trainium2 NeuronCores.

Math (reference): per-node k/v projections of x[N,512], per-head logits
s = (x@k_w.T + k_b) . q / sqrt(64), segment softmax over each graph's
nodes, weighted segment-sum of v, then o-projection + LayerNorm over
the [256, 512] graph outputs.

Key restructurings (all exact, up to fp reassociation):
  * Only k.q is needed, so the k-projection folds into W_s[512,8] =
    (k_w * q).heads.sum / 8 computed on host; s = x @ W_s.
  * Per-(graph,head) constants multiply both numerator and denominator
    of the softmax-average, so the segment-max subtraction and the
    k-bias term cancel -> w = exp(x @ W_s) directly (values are O(e^3),
    safe in fp32).
  * v-bias adds v_b * denom to the numerator -> attended = numer/denom
    + v_b; applied once in the tail.
  * The big reassociation: numer = segsum(w * (x @ v_w.T)) =
    (B.T @ x) @ v_w.T where B[n, (g,h)] = onehot[n,g] * w[n,h].
    Contracting nodes FIRST (256 output columns) costs half the PE work
    of projecting every node (512 columns), and the v-projection then
    runs once on the tiny [256, 512] aggregate in the tail.
  * B is built on-device from the segment ids (iota + is_equal) and the
    exp weights; the logits matmul runs transposed (s.T = W_s.T @ x.T)
    with a bf16 copy of x.T shipped from the host (softmax averaging
    washes out the bf16 logit rounding).

Sharding: batch is sorted, so core c owns graphs [32c, 32c+32) and their
contiguous node range, zero-padded to PAD_N.
"""

import numpy as np
from contextlib import ExitStack

import ml_dtypes
import concourse.bass as bass
import concourse.bacc as bacc
import concourse.tile as tile
from concourse import mybir
from concourse.bass_utils import run_bass_kernel_spmd
from concourse.masks import make_identity

N_CORES = 8
G = 256
G_LOC = G // N_CORES  # 32 graphs per core
H = 512
NH = 8
HD = 64
CHUNK = 512  # nodes per chunk
PAD_N = 13312  # 26 chunks; actual max per-core nodes is 12653 for this problem size
NCHUNK = PAD_N // CHUNK
KC = H // 128  # 4 contraction sub-chunks
JC = CHUNK // 128  # 4 node sub-chunks per chunk
GH = G_LOC * NH  # 256 (graph, head) columns
NHP = 16  # logits padded to 16 rows for the DMA-transpose xbar (src rows % 16)
LN_EPS = 1e-5

F32 = mybir.dt.float32
F32R = mybir.dt.float32r
BF16 = mybir.dt.bfloat16
FP16 = mybir.dt.float16


def _bcast_rows(ap_1d, parts):
    """[D] dram AP -> [parts, D] partition-broadcast AP (stride-0 partitions)."""
    return bass.AP(tensor=ap_1d.tensor, offset=ap_1d.offset, ap=[[0, parts]] + list(ap_1d.ap))


def _dup_inner(ap, n):
    """Append a 0-stride length-n innermost dim (free-dim broadcast)."""
    return bass.AP(tensor=ap.tensor, offset=ap.offset, ap=list(ap.ap) + [[0, n]])


def _dup_mid(ap, n):
    """Insert a 0-stride length-n dim before the innermost free dim."""
    aps = list(ap.ap)
    return bass.AP(tensor=ap.tensor, offset=ap.offset, ap=aps[:-1] + [[0, n]] + aps[-1:])


def build_bass():
    nc = bacc.Bacc(None)

    xn = nc.declare_dram_parameter("xn", [NCHUNK, CHUNK, H], F32R, isOutput=False)
    xtb = nc.declare_dram_parameter("xtb", [NCHUNK, H, CHUNK], FP16, isOutput=False)
    bt = nc.declare_dram_parameter("bt", [128, NCHUNK, JC], F32, isOutput=False)
    vwt = nc.declare_dram_parameter("vwt", [H, H], F32R, isOutput=False)
    wst = nc.declare_dram_parameter("wst", [H, NHP], FP16, isOutput=False)
    owt = nc.declare_dram_parameter("owt", [H, H], F32R, isOutput=False)
    vb = nc.declare_dram_parameter("vb", [H], F32, isOutput=False)
    ob = nc.declare_dram_parameter("ob", [H], F32, isOutput=False)
    lng = nc.declare_dram_parameter("lng", [H], F32, isOutput=False)
    lnb = nc.declare_dram_parameter("lnb", [H], F32, isOutput=False)
    out = nc.declare_dram_parameter("out", [G_LOC, H], F32, isOutput=True)

    xn_r = xn.rearrange("c (j p) f -> c p j f", p=128)
    xtb_r = xtb.rearrange("c (k p) n -> c p k n", p=128)
    vwt_r = vwt.rearrange("(k p) o -> p k o", p=128)
    wst_r = wst.rearrange("(k p) h -> p k h", p=128)
    owt_r = owt.rearrange("(k p) o -> p k o", p=128)

    with tile.TileContext(nc) as tc, ExitStack() as ctx:
        consts = ctx.enter_context(tc.tile_pool(name="consts", bufs=1))
        acc_psum = ctx.enter_context(tc.tile_pool(name="acc_psum", bufs=1, space="PSUM"))

        # --- constants (loaded once) ---
        vwt_sb = consts.tile([128, KC, H], F32R)
        nc.sync.dma_start(out=vwt_sb, in_=vwt_r)
        wst_sb = consts.tile([128, KC, NHP], FP16)
        nc.sync.dma_start(out=wst_sb, in_=wst_r)
        owt_sb = consts.tile([128, KC, H], F32R)
        nc.sync.dma_start(out=owt_sb, in_=owt_r)
        bt_sb = consts.tile([128, NCHUNK, JC], F32)
        nc.sync.dma_start(out=bt_sb, in_=bt[:, :, :])
        vb_sb = consts.tile([G_LOC, H], F32)
        nc.gpsimd.dma_start(out=vb_sb, in_=_bcast_rows(vb[:], G_LOC))
        ob_sb = consts.tile([G_LOC, H], F32)
        nc.gpsimd.dma_start(out=ob_sb, in_=_bcast_rows(ob[:], G_LOC))
        lng_sb = consts.tile([G_LOC, H], F32)
        nc.gpsimd.dma_start(out=lng_sb, in_=_bcast_rows(lng[:], G_LOC))
        lnb_sb = consts.tile([G_LOC, H], F32)
        nc.gpsimd.dma_start(out=lnb_sb, in_=_bcast_rows(lnb[:], G_LOC))

        iota_g = consts.tile([128, G_LOC], F32)
        nc.gpsimd.iota(iota_g, pattern=[[1, G_LOC]], base=0, channel_multiplier=0,
                       allow_small_or_imprecise_dtypes=True)
        identity = consts.tile([128, 128], F32)
        make_identity(nc, identity)
        eps_t = consts.tile([G_LOC, 1], F32)
        nc.vector.memset(eps_t, LN_EPS)

        # cross-chunk accumulators: C = B.T @ x (two 128-col halves), denom
        c0_ps = acc_psum.tile([128, H], F32)
        c1_ps = acc_psum.tile([128, H], F32)
        denom_ps = acc_psum.tile([G_LOC, NH], F32)
        c_halves = [c0_ps, c1_ps]

        # --- main loop over node chunks ---
        with ExitStack() as loop_ctx:
            xn_pool = loop_ctx.enter_context(tc.tile_pool(name="xn", bufs=3))
            xtb_pool = loop_ctx.enter_context(tc.tile_pool(name="xtb", bufs=3))
            work = loop_ctx.enter_context(tc.tile_pool(name="work", bufs=2))
            sT_pool = loop_ctx.enter_context(tc.tile_pool(name="sT_ps", bufs=2, space="PSUM"))
            sn_pool = loop_ctx.enter_context(tc.tile_pool(name="sn_ps", bufs=2, space="PSUM"))

            def emit_s_matmuls(i):
                # s.T = W_s.T @ x.T : one [8, 512] fp16 matmul chain per chunk,
                # emitted one chunk ahead so the PSUM->SBUF copy overlaps C-work
                xtb_sb = xtb_pool.tile([128, KC, CHUNK], FP16, tag="xtb")
                nc.sync.dma_start(out=xtb_sb, in_=xtb_r[i])
                sT_ps = sT_pool.tile([NHP, CHUNK], F32, tag="sT")
                sT_sb = work.tile([NHP, CHUNK], F32, tag="sTsb")
                for k in range(KC):
                    nc.tensor.matmul(sT_ps, lhsT=wst_sb[:, k, :], rhs=xtb_sb[:, k, :],
                                     start=(k == 0), stop=(k == KC - 1))
                nc.scalar.copy(sT_sb, sT_ps)
                return sT_sb

            sT_queue = [emit_s_matmuls(0), emit_s_matmuls(1)]
            for i in range(NCHUNK):
                xn_sb = xn_pool.tile([128, JC, H], F32R)
                nc.sync.dma_start(out=xn_sb, in_=xn_r[i])

                at_sb = work.tile([128, JC, G_LOC], F32R)
                for j in range(JC):
                    b1 = bt_sb[:, i, j:j + 1]
                    bt_b = bass.AP(tensor=b1.tensor, offset=b1.offset,
                                   ap=[list(b1.ap[0])] + [[0, G_LOC]])
                    nc.vector.tensor_tensor(out=at_sb[:, j, :], in0=iota_g, in1=bt_b,
                                            op=mybir.AluOpType.is_equal)

                sT_sb = sT_queue.pop(0)
                sn_ps = sn_pool.tile([128, JC, NH], F32)
                w_n = work.tile([128, JC, NH], F32R)
                bmat = work.tile([128, JC, G_LOC, NH], F32R)

                # PE: transposes first (sT copy landed during last chunk's C),
                # then next chunk's s-matmuls, then this chunk's C matmuls.
                for j in range(JC):
                    nc.tensor.transpose(sn_ps[:, j, :], sT_sb[:NH, bass.ts(j, 128)],
                                        identity[:NH, :NH])
                for j in range(JC):
                    nc.scalar.activation(w_n[:, j, :], sn_ps[:, j, :],
                                         mybir.ActivationFunctionType.Exp)
                    # B[n, (g,h)] = at[n,g] * w[n,h]
                    nc.vector.tensor_tensor(
                        out=bmat[:, j, :, :],
                        in0=_dup_inner(at_sb[:, j, :], NH),
                        in1=_dup_mid(w_n[:, j, :], G_LOC),
                        op=mybir.AluOpType.mult)

                if i + 2 < NCHUNK:
                    sT_queue.append(emit_s_matmuls(i + 2))

                for j in range(JC):
                    bflat = bmat[:, j, :, :].rearrange("p g h -> p (g h)")
                    for half in range(2):
                        nc.tensor.matmul(
                            c_halves[half], lhsT=bflat[:, bass.ts(half, 128)],
                            rhs=xn_sb[:, j, :],
                            start=(i == 0 and j == 0), stop=(i == NCHUNK - 1 and j == JC - 1),
                            skip_group_check=True)
                    nc.tensor.matmul(
                        denom_ps, lhsT=at_sb[:, j, :], rhs=w_n[:, j, :],
                        start=(i == 0 and j == 0), stop=(i == NCHUNK - 1 and j == JC - 1),
                        skip_group_check=True)

        # --- tail: C -> C.T -> numer -> attended -> o-projection -> layernorm ---
        with ExitStack() as tail_ctx:
            tail = tail_ctx.enter_context(tc.tile_pool(name="tail", bufs=1))
            tail_ps = tail_ctx.enter_context(tc.tile_pool(name="tail_ps", bufs=2, space="PSUM"))
            acc2_ps = tail_ctx.enter_context(tc.tile_pool(name="acc2_ps", bufs=1, space="PSUM"))

            c_sb = tail.tile([128, 2, H], F32)
            nc.vector.tensor_copy(c_sb[:, 0, :], c0_ps)
            nc.scalar.copy(c_sb[:, 1, :], c1_ps)

            # transpose C [256(gh), 512(i)] -> CT [512(i), 256(gh)]
            ct_sb = tail.tile([128, KC, GH], F32R)
            for k in range(KC):
                for half in range(2):
                    ctp = tail_ps.tile([128, 128], F32, tag="ctp")
                    nc.tensor.transpose(ctp, c_sb[:, half, bass.ts(k, 128)],
                                        identity[:, :])
                    eng = nc.vector.tensor_copy if half == 0 else nc.scalar.copy
                    eng(ct_sb[:, k, bass.ts(half, 128)], ctp)

            # numer[(g,h), :] = CT_h.T @ vwt[:, h*64:(h+1)*64]
            numer_ps = acc2_ps.tile([G_LOC, NH, HD], F32)
            ct_ghview = ct_sb.rearrange("p k (g h) -> p k g h", h=NH)
            for h in range(NH):
                for k in range(KC):
                    nc.tensor.matmul(
                        numer_ps[:, h, :], lhsT=ct_ghview[:, k, :, h],
                        rhs=vwt_sb[:, k, bass.ts(h, HD)],
                        start=(k == 0), stop=(k == KC - 1))

            recip = tail.tile([G_LOC, NH], F32)
            nc.vector.reciprocal(recip, denom_ps)

            att = tail.tile([G_LOC, NH, HD], F32)
            nc.vector.tensor_tensor(out=att[:, :, :],
                                    in0=numer_ps[:, :, :],
                                    in1=_dup_inner(recip[:, :], HD),
                                    op=mybir.AluOpType.mult)
            att_f = att.rearrange("g h d -> g (h d)")
            nc.vector.tensor_add(att_f, att_f, vb_sb)

            attT_ps = acc2_ps.tile([128, KC, G_LOC], F32)
            attT_sb = tail.tile([128, KC, G_LOC], F32R)
            for k in range(KC):
                nc.tensor.transpose(attT_ps[:, k, :], att_f[:, bass.ts(k, 128)],
                                    identity[:G_LOC, :G_LOC])
            nc.scalar.copy(attT_sb.rearrange("p k g -> p (k g)"),
                           attT_ps.rearrange("p k g -> p (k g)"))

            oo_ps = acc2_ps.tile([G_LOC, H], F32)
            for k in range(KC):
                nc.tensor.matmul(oo_ps, lhsT=attT_sb[:, k, :], rhs=owt_sb[:, k, :],
                                 start=(k == 0), stop=(k == KC - 1))

            t_sb = tail.tile([G_LOC, H], F32)
            nc.vector.tensor_add(t_sb, oo_ps, ob_sb)
            stats = tail.tile([G_LOC, 6], F32)
            nc.vector.bn_stats(out=stats, in_=t_sb)
            mv = tail.tile([G_LOC, 2], F32)
            nc.vector.bn_aggr(out=mv, in_=stats)
            std = tail.tile([G_LOC, 1], F32)
            nc.scalar.activation(std, mv[:, 1:2], mybir.ActivationFunctionType.Sqrt,
                                 bias=eps_t[:, 0:1])
            rstd = tail.tile([G_LOC, 1], F32)
            nc.vector.reciprocal(rstd, std)

            m1 = mv[:, 0:1]
            mean_b = bass.AP(tensor=m1.tensor, offset=m1.offset,
                             ap=[list(m1.ap[0])] + [[0, H]])
            r1 = rstd[:, 0:1]
            rstd_b = bass.AP(tensor=r1.tensor, offset=r1.offset,
                             ap=[list(r1.ap[0])] + [[0, H]])
            t2 = tail.tile([G_LOC, H], F32)
            nc.vector.tensor_sub(t2, t_sb, mean_b)
            nc.vector.tensor_mul(t2, t2, rstd_b)
            nc.vector.tensor_mul(t2, t2, lng_sb)
            out_sb = tail.tile([G_LOC, H], F32)
            nc.vector.tensor_add(out_sb, t2, lnb_sb)
            nc.sync.dma_start(out=out[:, :], in_=out_sb)

    nc.finalize()
    return nc


_NC_CACHE = {}


def _get_nc():
    if "nc" not in _NC_CACHE:
        _NC_CACHE["nc"] = build_bass()
    return _NC_CACHE["nc"]


def kernel(x, batch, query, k_w, k_b, v_w, v_b, o_w, o_b, ln_g, ln_b, num_graphs):
    x = np.ascontiguousarray(np.asarray(x, dtype=np.float32))
    batch = np.asarray(batch).astype(np.int64)
    query = np.asarray(query, dtype=np.float32)
    k_w = np.asarray(k_w, dtype=np.float32)
    v_w = np.asarray(v_w, dtype=np.float32)
    o_w = np.asarray(o_w, dtype=np.float32)
    v_b = np.asarray(v_b, dtype=np.float32)
    o_b = np.asarray(o_b, dtype=np.float32)
    ln_g = np.asarray(ln_g, dtype=np.float32)
    ln_b = np.asarray(ln_b, dtype=np.float32)

    # fold k-projection + query dot + 1/sqrt(hd) into one [H, NH] matrix
    w_s = np.zeros((H, NHP), np.float16)
    w_s[:, :NH] = ((k_w * query[:, None]).reshape(NH, HD, H).sum(1).T
                   / np.float32(np.sqrt(HD))).astype(np.float16)
    vwt_h = np.ascontiguousarray(v_w.T)
    owt_h = np.ascontiguousarray(o_w.T)

    bounds = np.searchsorted(batch, np.arange(0, G + 1, G_LOC))
    in_maps = []
    for c in range(N_CORES):
        s, e = int(bounds[c]), int(bounds[c + 1])
        n_loc = e - s
        assert n_loc <= PAD_N, f"shard {c} has {n_loc} nodes > PAD_N={PAD_N}"
        xs = np.zeros((PAD_N, H), np.float32)
        xs[:n_loc] = x[s:e]
        xn_h = np.ascontiguousarray(xs.reshape(NCHUNK, CHUNK, H))
        xtb_h = np.ascontiguousarray(
            xs.reshape(NCHUNK, CHUNK, H).transpose(0, 2, 1)).astype(np.float16)
        bl = np.full(PAD_N, -1.0, np.float32)
        bl[:n_loc] = (batch[s:e] - G_LOC * c).astype(np.float32)
        bt_h = np.ascontiguousarray(bl.reshape(NCHUNK, JC, 128).transpose(2, 0, 1))
        in_maps.append(dict(xn=xn_h, xtb=xtb_h, bt=bt_h, vwt=vwt_h, wst=w_s, owt=owt_h,
                            vb=v_b, ob=o_b, lng=ln_g, lnb=ln_b))

    nc = _get_nc()
    res = run_bass_kernel_spmd(nc, in_maps, core_ids=list(range(N_CORES)))
    _NC_CACHE["last_results"] = res
    return np.concatenate([res.results[c]["out"] for c in range(N_CORES)], axis=0)


# revision 21
# speedup vs baseline: 2.7918x; 1.5301x over previous
"""AttentionReadout kernel for 8 Trainium2 NeuronCores.

Math (reference): per-node k/v projections of x[N,512], per-head logits
s = (x@k_w.T + k_b) . q / sqrt(64), segment softmax over each graph's
nodes, weighted segment-sum of v, then o-projection + LayerNorm over
the [256, 512] graph outputs.

Key restructurings (all exact, up to fp reassociation):
  * Only k.q is needed, so the k-projection folds into W_s[512,8] =
    (k_w * q).heads.sum / 8 computed on host; s = x @ W_s.
  * Per-(graph,head) constants multiply both numerator and denominator
    of the softmax-average, so the segment-max subtraction and the
    k-bias term cancel -> w = exp(x @ W_s) directly (values are O(e^3),
    safe in fp32).
  * v-bias adds v_b * denom to the numerator -> attended = numer/denom
    + v_b; applied once in the tail.
  * The big reassociation: numer = segsum(w * (x @ v_w.T)) =
    (B.T @ x) @ v_w.T where B[n, (g,h)] = onehot[n,g] * w[n,h].
    Contracting nodes FIRST (256 output columns) costs half the PE work
    of projecting every node (512 columns), and the v-projection then
    runs once on the tiny [256, 512] aggregate in the tail.
  * B is built on-device from the segment ids (iota + is_equal) and the
    exp weights; the logits matmul runs transposed (s.T = W_s.T @ x.T)
    with a bf16 copy of x.T shipped from the host (softmax averaging
    washes out the bf16 logit rounding).

Sharding: batch is sorted, so core c owns graphs [32c, 32c+32) and their
contiguous node range, zero-padded to PAD_N.
"""

import numpy as np
from contextlib import ExitStack

import ml_dtypes
import concourse.bass as bass
import concourse.bacc as bacc
import concourse.tile as tile
from concourse import mybir
from concourse.bass_utils import run_bass_kernel_spmd
from concourse.masks import make_identity

N_CORES = 8
G = 256
G_LOC = G // N_CORES  # 32 graphs per core
H = 512
NH = 8
HD = 64
CHUNK = 512  # nodes per chunk
PAD_N = 13312  # 26 chunks; actual max per-core nodes is 12653 for this problem size
NCHUNK = PAD_N // CHUNK
KC = H // 128  # 4 contraction sub-chunks
JC = CHUNK // 128  # 4 node sub-chunks per chunk
GH = G_LOC * NH  # 256 (graph, head) columns
NHP = 16  # logits padded to 16 rows for the DMA-transpose xbar (src rows % 16)
LN_EPS = 1e-5

F32 = mybir.dt.float32
F32R = mybir.dt.float32r
BF16 = mybir.dt.bfloat16
FP16 = mybir.dt.float16


def _bcast_rows(ap_1d, parts):
    """[D] dram AP -> [parts, D] partition-broadcast AP (stride-0 partitions)."""
    return bass.AP(tensor=ap_1d.tensor, offset=ap_1d.offset, ap=[[0, parts]] + list(ap_1d.ap))


def _dup_inner(ap, n):
    """Append a 0-stride length-n innermost dim (free-dim broadcast)."""
    return bass.AP(tensor=ap.tensor, offset=ap.offset, ap=list(ap.ap) + [[0, n]])


def _dup_mid(ap, n):
    """Insert a 0-stride length-n dim before the innermost free dim."""
    aps = list(ap.ap)
    return bass.AP(tensor=ap.tensor, offset=ap.offset, ap=aps[:-1] + [[0, n]] + aps[-1:])


def build_bass():
    nc = bacc.Bacc(None)

    xn = nc.declare_dram_parameter("xn", [NCHUNK, CHUNK, H], FP16, isOutput=False)
    xtb = nc.declare_dram_parameter("xtb", [NCHUNK, H, CHUNK], FP16, isOutput=False)
    bt = nc.declare_dram_parameter("bt", [128, NCHUNK, JC], F32, isOutput=False)
    vwt = nc.declare_dram_parameter("vwt", [H, H], F32R, isOutput=False)
    wst = nc.declare_dram_parameter("wst", [H, NHP], FP16, isOutput=False)
    owt = nc.declare_dram_parameter("owt", [H, H], F32R, isOutput=False)
    vb = nc.declare_dram_parameter("vb", [H], F32, isOutput=False)
    ob = nc.declare_dram_parameter("ob", [H], F32, isOutput=False)
    lng = nc.declare_dram_parameter("lng", [H], F32, isOutput=False)
    lnb = nc.declare_dram_parameter("lnb", [H], F32, isOutput=False)
    out = nc.declare_dram_parameter("out", [G_LOC, H], F32, isOutput=True)

    xn_r = xn.rearrange("c (j p) f -> c p j f", p=128)
    xtb_r = xtb.rearrange("c (k p) n -> c p k n", p=128)
    vwt_r = vwt.rearrange("(k p) o -> p k o", p=128)
    wst_r = wst.rearrange("(k p) h -> p k h", p=128)
    owt_r = owt.rearrange("(k p) o -> p k o", p=128)

    with tile.TileContext(nc) as tc, ExitStack() as ctx:
        consts = ctx.enter_context(tc.tile_pool(name="consts", bufs=1))
        acc_psum = ctx.enter_context(tc.tile_pool(name="acc_psum", bufs=1, space="PSUM"))

        # --- constants (loaded once) ---
        wst_sb = consts.tile([128, KC, NHP], FP16)
        nc.sync.dma_start(out=wst_sb, in_=wst_r)
        vwt_sb = consts.tile([128, KC, H], F32R)
        owt_sb = consts.tile([128, KC, H], F32R)
        bt_sb = consts.tile([128, NCHUNK, JC], F32)
        nc.sync.dma_start(out=bt_sb, in_=bt[:, :, :])
        vb_sb = consts.tile([G_LOC, H], F32)
        nc.gpsimd.dma_start(out=vb_sb, in_=_bcast_rows(vb[:], G_LOC))
        ob_sb = consts.tile([G_LOC, H], F32)
        nc.gpsimd.dma_start(out=ob_sb, in_=_bcast_rows(ob[:], G_LOC))
        lng_sb = consts.tile([G_LOC, H], F32)
        nc.gpsimd.dma_start(out=lng_sb, in_=_bcast_rows(lng[:], G_LOC))
        lnb_sb = consts.tile([G_LOC, H], F32)
        nc.gpsimd.dma_start(out=lnb_sb, in_=_bcast_rows(lnb[:], G_LOC))

        iota_g = consts.tile([128, G_LOC], F32)
        nc.gpsimd.iota(iota_g, pattern=[[1, G_LOC]], base=0, channel_multiplier=0,
                       allow_small_or_imprecise_dtypes=True)
        identity = consts.tile([128, 128], F32)
        make_identity(nc, identity)
        eps_t = consts.tile([G_LOC, 1], F32)
        nc.vector.memset(eps_t, LN_EPS)

        # cross-chunk accumulators: C = B.T @ x (two 128-col halves), denom
        c0_ps = acc_psum.tile([128, H], F32)
        c1_ps = acc_psum.tile([128, H], F32)
        denom_ps = acc_psum.tile([G_LOC, NH], F32)
        c_halves = [c0_ps, c1_ps]

        # --- main loop over node chunks ---
        with ExitStack() as loop_ctx:
            xn_pool = loop_ctx.enter_context(tc.tile_pool(name="xn", bufs=3))
            xtb_pool = loop_ctx.enter_context(tc.tile_pool(name="xtb", bufs=3))
            work = loop_ctx.enter_context(tc.tile_pool(name="work", bufs=2))
            sT_pool = loop_ctx.enter_context(tc.tile_pool(name="sT_ps", bufs=2, space="PSUM"))
            sn_pool = loop_ctx.enter_context(tc.tile_pool(name="sn_ps", bufs=2, space="PSUM"))

            def emit_s_matmuls(i):
                # s.T = W_s.T @ x.T : one [8, 512] fp16 matmul chain per chunk,
                # emitted one chunk ahead so the PSUM->SBUF copy overlaps C-work
                xtb_sb = xtb_pool.tile([128, KC, CHUNK], FP16, tag="xtb")
                nc.sync.dma_start(out=xtb_sb, in_=xtb_r[i])
                sT_ps = sT_pool.tile([NHP, CHUNK], F32, tag="sT")
                sT_sb = work.tile([NHP, CHUNK], F32, tag="sTsb")
                for k in range(KC):
                    nc.tensor.matmul(sT_ps, lhsT=wst_sb[:, k, :], rhs=xtb_sb[:, k, :],
                                     start=(k == 0), stop=(k == KC - 1))
                nc.scalar.copy(sT_sb, sT_ps)
                return sT_sb

            sT_queue = [emit_s_matmuls(0), emit_s_matmuls(1)]
            for i in range(NCHUNK):
                xn_sb = xn_pool.tile([128, JC, H], FP16)
                nc.sync.dma_start(out=xn_sb, in_=xn_r[i])

                at_sb = work.tile([128, JC, G_LOC], FP16)
                for j in range(JC):
                    b1 = bt_sb[:, i, j:j + 1]
                    bt_b = bass.AP(tensor=b1.tensor, offset=b1.offset,
                                   ap=[list(b1.ap[0])] + [[0, G_LOC]])
                    nc.vector.tensor_tensor(out=at_sb[:, j, :], in0=iota_g, in1=bt_b,
                                            op=mybir.AluOpType.is_equal)

                sT_sb = sT_queue.pop(0)
                sn_ps = sn_pool.tile([128, JC, NH], F32)
                w_n = work.tile([128, JC, NH], FP16)
                bmat = work.tile([128, JC, G_LOC, NH], FP16)

                # PE: transposes first (sT copy landed during last chunk's C),
                # then next chunk's s-matmuls, then this chunk's C matmuls.
                for j in range(JC):
                    nc.tensor.transpose(sn_ps[:, j, :], sT_sb[:NH, bass.ts(j, 128)],
                                        identity[:NH, :NH])
                for j in range(JC):
                    nc.scalar.activation(w_n[:, j, :], sn_ps[:, j, :],
                                         mybir.ActivationFunctionType.Exp)
                    # B[n, (g,h)] = at[n,g] * w[n,h]
                    nc.vector.tensor_tensor(
                        out=bmat[:, j, :, :],
                        in0=_dup_inner(at_sb[:, j, :], NH),
                        in1=_dup_mid(w_n[:, j, :], G_LOC),
                        op=mybir.AluOpType.mult)

                if i + 2 < NCHUNK:
                    sT_queue.append(emit_s_matmuls(i + 2))

                for j in range(JC):
                    bflat = bmat[:, j, :, :].rearrange("p g h -> p (g h)")
                    for half in range(2):
                        nc.tensor.matmul(
                            c_halves[half], lhsT=bflat[:, bass.ts(half, 128)],
                            rhs=xn_sb[:, j, :],
                            start=(i == 0 and j == 0), stop=(i == NCHUNK - 1 and j == JC - 1),
                            skip_group_check=True)
                    nc.tensor.matmul(
                        denom_ps, lhsT=at_sb[:, j, :], rhs=w_n[:, j, :],
                        start=(i == 0 and j == 0), stop=(i == NCHUNK - 1 and j == JC - 1),
                        skip_group_check=True)

        # tail-only projection weights: loaded late so chunk DMA wins the queues
        nc.sync.dma_start(out=vwt_sb, in_=vwt_r)
        nc.sync.dma_start(out=owt_sb, in_=owt_r)

        # --- tail: C -> C.T -> numer -> attended -> o-projection -> layernorm ---
        with ExitStack() as tail_ctx:
            tail = tail_ctx.enter_context(tc.tile_pool(name="tail", bufs=1))
            tail_ps = tail_ctx.enter_context(tc.tile_pool(name="tail_ps", bufs=2, space="PSUM"))
            acc2_ps = tail_ctx.enter_context(tc.tile_pool(name="acc2_ps", bufs=1, space="PSUM"))

            c_sb = tail.tile([128, 2, H], F32)
            nc.vector.tensor_copy(c_sb[:, 0, :], c0_ps)
            nc.scalar.copy(c_sb[:, 1, :], c1_ps)

            # transpose C [256(gh), 512(i)] -> CT [512(i), 256(gh)]
            ct_sb = tail.tile([128, KC, GH], F32R)
            for k in range(KC):
                for half in range(2):
                    ctp = tail_ps.tile([128, 128], F32, tag="ctp")
                    nc.tensor.transpose(ctp, c_sb[:, half, bass.ts(k, 128)],
                                        identity[:, :])
                    eng = nc.vector.tensor_copy if half == 0 else nc.scalar.copy
                    eng(ct_sb[:, k, bass.ts(half, 128)], ctp)

            # numer[(g,h), :] = CT_h.T @ vwt[:, h*64:(h+1)*64]
            numer_ps = acc2_ps.tile([G_LOC, NH, HD], F32)
            ct_ghview = ct_sb.rearrange("p k (g h) -> p k g h", h=NH)
            for h in range(NH):
                for k in range(KC):
                    nc.tensor.matmul(
                        numer_ps[:, h, :], lhsT=ct_ghview[:, k, :, h],
                        rhs=vwt_sb[:, k, bass.ts(h, HD)],
                        start=(k == 0), stop=(k == KC - 1))

            recip = tail.tile([G_LOC, NH], F32)
            nc.vector.reciprocal(recip, denom_ps)

            att = tail.tile([G_LOC, NH, HD], F32)
            nc.vector.tensor_tensor(out=att[:, :, :],
                                    in0=numer_ps[:, :, :],
                                    in1=_dup_inner(recip[:, :], HD),
                                    op=mybir.AluOpType.mult)
            att_f = att.rearrange("g h d -> g (h d)")
            nc.vector.tensor_add(att_f, att_f, vb_sb)

            attT_ps = acc2_ps.tile([128, KC, G_LOC], F32)
            attT_sb = tail.tile([128, KC, G_LOC], F32R)
            for k in range(KC):
                nc.tensor.transpose(attT_ps[:, k, :], att_f[:, bass.ts(k, 128)],
                                    identity[:G_LOC, :G_LOC])
            nc.scalar.copy(attT_sb.rearrange("p k g -> p (k g)"),
                           attT_ps.rearrange("p k g -> p (k g)"))

            oo_ps = acc2_ps.tile([G_LOC, H], F32)
            for k in range(KC):
                nc.tensor.matmul(oo_ps, lhsT=attT_sb[:, k, :], rhs=owt_sb[:, k, :],
                                 start=(k == 0), stop=(k == KC - 1))

            t_sb = tail.tile([G_LOC, H], F32)
            nc.vector.tensor_add(t_sb, oo_ps, ob_sb)
            stats = tail.tile([G_LOC, 6], F32)
            nc.vector.bn_stats(out=stats, in_=t_sb)
            mv = tail.tile([G_LOC, 2], F32)
            nc.vector.bn_aggr(out=mv, in_=stats)
            std = tail.tile([G_LOC, 1], F32)
            nc.scalar.activation(std, mv[:, 1:2], mybir.ActivationFunctionType.Sqrt,
                                 bias=eps_t[:, 0:1])
            rstd = tail.tile([G_LOC, 1], F32)
            nc.vector.reciprocal(rstd, std)

            m1 = mv[:, 0:1]
            mean_b = bass.AP(tensor=m1.tensor, offset=m1.offset,
                             ap=[list(m1.ap[0])] + [[0, H]])
            r1 = rstd[:, 0:1]
            rstd_b = bass.AP(tensor=r1.tensor, offset=r1.offset,
                             ap=[list(r1.ap[0])] + [[0, H]])
            t2 = tail.tile([G_LOC, H], F32)
            nc.vector.tensor_sub(t2, t_sb, mean_b)
            nc.vector.tensor_mul(t2, t2, rstd_b)
            nc.vector.tensor_mul(t2, t2, lng_sb)
            out_sb = tail.tile([G_LOC, H], F32)
            nc.vector.tensor_add(out_sb, t2, lnb_sb)
            nc.sync.dma_start(out=out[:, :], in_=out_sb)

    nc.finalize()
    return nc


_NC_CACHE = {}


def _get_nc():
    if "nc" not in _NC_CACHE:
        _NC_CACHE["nc"] = build_bass()
    return _NC_CACHE["nc"]


def kernel(x, batch, query, k_w, k_b, v_w, v_b, o_w, o_b, ln_g, ln_b, num_graphs):
    x = np.ascontiguousarray(np.asarray(x, dtype=np.float32))
    batch = np.asarray(batch).astype(np.int64)
    query = np.asarray(query, dtype=np.float32)
    k_w = np.asarray(k_w, dtype=np.float32)
    v_w = np.asarray(v_w, dtype=np.float32)
    o_w = np.asarray(o_w, dtype=np.float32)
    v_b = np.asarray(v_b, dtype=np.float32)
    o_b = np.asarray(o_b, dtype=np.float32)
    ln_g = np.asarray(ln_g, dtype=np.float32)
    ln_b = np.asarray(ln_b, dtype=np.float32)

    # fold k-projection + query dot + 1/sqrt(hd) into one [H, NH] matrix
    w_s = np.zeros((H, NHP), np.float16)
    w_s[:, :NH] = ((k_w * query[:, None]).reshape(NH, HD, H).sum(1).T
                   / np.float32(np.sqrt(HD))).astype(np.float16)
    vwt_h = np.ascontiguousarray(v_w.T)
    owt_h = np.ascontiguousarray(o_w.T)

    bounds = np.searchsorted(batch, np.arange(0, G + 1, G_LOC))
    in_maps = []
    for c in range(N_CORES):
        s, e = int(bounds[c]), int(bounds[c + 1])
        n_loc = e - s
        assert n_loc <= PAD_N, f"shard {c} has {n_loc} nodes > PAD_N={PAD_N}"
        xs = np.zeros((PAD_N, H), np.float32)
        xs[:n_loc] = x[s:e]
        xn_h = np.ascontiguousarray(xs.reshape(NCHUNK, CHUNK, H)).astype(np.float16)
        xtb_h = np.ascontiguousarray(
            xs.reshape(NCHUNK, CHUNK, H).transpose(0, 2, 1)).astype(np.float16)
        bl = np.full(PAD_N, -1.0, np.float32)
        bl[:n_loc] = (batch[s:e] - G_LOC * c).astype(np.float32)
        bt_h = np.ascontiguousarray(bl.reshape(NCHUNK, JC, 128).transpose(2, 0, 1))
        in_maps.append(dict(xn=xn_h, xtb=xtb_h, bt=bt_h, vwt=vwt_h, wst=w_s, owt=owt_h,
                            vb=v_b, ob=o_b, lng=ln_g, lnb=ln_b))

    nc = _get_nc()
    res = run_bass_kernel_spmd(nc, in_maps, core_ids=list(range(N_CORES)))
    _NC_CACHE["last_results"] = res
    return np.concatenate([res.results[c]["out"] for c in range(N_CORES)], axis=0)
